# revision 58
# baseline (speedup 1.0000x reference)
"""Trainium2 Bass kernel for nn_BiAttention (sparse_attention).

Math: the attention matrix is rank-1 plus a mask bias:
    att[b,l,m] = idot[b,l] + s_m[b,m]
Row softmax over m is invariant to the per-row constant idot[b,l], so
    output_one[b,l,:] = v_b := softmax_m(s_m) @ (memory @ W_mem2.T + b_mem2)
and max_m att = idot + const, so
    output_two[b,0,:] = softmax_l(idot) @ inp2 = (W_in2 @ q + Z*b_in2)/Z
    with q[d] = sum_l e_l * input[l,d],  Z = sum_l e_l,  e = exp(idot)
Output row blocks [N, 4*Ld, d]:
    [0:2048]    inp2 = input @ W_in2.T + b_in2        (full rank, device)
    [2048:4096] v_b broadcast                          (host replicates row)
    [4096:6144] inp2 * v_b                             (full rank, device)
    [6144:8192] (output_two * v_b) broadcast           (host replicates row)

Device computes everything TRANSPOSED (output features on partitions):
    i2T[o, l] = inp2[l, o] = sum_d W_in2^T[d,o] * input^T[d,l] + b[o]
so the bias is a per-partition scalar (ACT engine Identity+bias) and
prod = i2T * v[o] is a per-partition tensor_scalar on DVE. All matmul
operands are bf16 (inputs/weights pre-transposed + converted on host;
the PE program must be single-dtype: mixing bf16 with f32r transposes
faults the exec unit). Outputs stored bf16, upconverted on host:
rel err ~5e-3 << 2e-2.

idot rides as a [1,512] row accumulated over k with a [128,1] stationary
(w_in1 column), which makes e_bc construction transpose-free: idot row ->
K=1 broadcast matmul -> Exp (with accumulator for Z) straight into a
[128, 2048] replicated tile that serves both the q reduction (d on
partitions) and nothing else.

Sharding: pure data parallel, one batch element per NeuronCore (8 cores).

Scheduling: engine queues are strict in-order; emission order is the
schedule. mm tiles t=1..32 (g outer, oc inner); side events are spliced
between tiles no earlier than their producers. Reads are split between
the ACT ring (early: w2t/input groups 0-1) and the Pool SWDGE ring
(late: groups 2-3, memory, W_mem2) so no sequencer stalls compute;
writes own the SP ring.
"""

import numpy as np
import ml_dtypes

import concourse.bass as bass
import concourse.tile as tile
from concourse import bacc, bass_isa, mybir
from concourse.bass_utils import run_bass_kernel_spmd

F32 = mybir.dt.float32
BF16 = mybir.dt.bfloat16
AX = mybir.AxisListType
OP = mybir.AluOpType
EXP = mybir.ActivationFunctionType.Exp
IDENT = mybir.ActivationFunctionType.Identity
COPY = mybir.ActivationFunctionType.Copy

P = 128
BSZ, LD, LM, HID = 8, 2048, 512, 1024
KT = HID // P          # 8 hidden-dim chunks
LT = LD // P           # 16 l slices of 128
MT = LM // P           # 4 memory tiles
GT = 4                 # l groups of 512
GL = LD // GT          # 512
N_CORES = 8

_NC_CACHE = None


def _build_nc():
    nc = bacc.Bacc("TRN2", target_bir_lowering=False, num_devices=N_CORES)

    inT_d = nc.dram_tensor("inT", [HID, LD], BF16, kind="ExternalInput").ap()
    w2t_d = nc.dram_tensor("w2t", [HID, HID], BF16, kind="ExternalInput").ap()
    wm2t_d = nc.dram_tensor("wm2t", [HID, HID], BF16, kind="ExternalInput").ap()
    mem_d = nc.dram_tensor("memory", [LM, HID], BF16, kind="ExternalInput").ap()
    mask_d = nc.dram_tensor("mask", [1, LM], F32, kind="ExternalInput").ap()
    w1c_d = nc.dram_tensor("w1c", [P, KT], BF16, kind="ExternalInput").ap()
    wm1_d = nc.dram_tensor("wm1", [1, HID], BF16, kind="ExternalInput").ap()
    bi2c_d = nc.dram_tensor("bi2c", [P, KT], F32, kind="ExternalInput").ap()
    bm2c_d = nc.dram_tensor("bm2c", [P, KT], F32, kind="ExternalInput").ap()
    o1T_d = nc.dram_tensor("o1T", [HID, LD], BF16, kind="ExternalOutput").ap()
    o3T_d = nc.dram_tensor("o3T", [HID, LD], BF16, kind="ExternalOutput").ap()
    vrow_d = nc.dram_tensor("vrow", [P, KT], F32, kind="ExternalOutput").ap()
    urow_d = nc.dram_tensor("urow", [P, KT], F32, kind="ExternalOutput").ap()

    with tile.TileContext(nc) as tc:
        with (
            tc.tile_pool(name="const", bufs=1) as cpool,
            tc.tile_pool(name="w", bufs=1) as wpool,
            tc.tile_pool(name="inp", bufs=1) as inpool,
            tc.tile_pool(name="i2", bufs=1) as i2pool,
            tc.tile_pool(name="o3", bufs=1) as o3pool,
            tc.tile_pool(name="scr", bufs=3) as scrpool,
            tc.tile_pool(name="sm", bufs=1) as smpool,
            tc.tile_pool(name="psmm", bufs=4, space="PSUM") as psmm,
            tc.tile_pool(name="psbc", bufs=1, space="PSUM") as psbc,
            tc.tile_pool(name="psid", bufs=1, space="PSUM") as psid,
            tc.tile_pool(name="pssm", bufs=2, space="PSUM") as pssm,
        ):
            # ---------------- constants ----------------
            ones_rowb = cpool.tile([1, P], BF16)   # partition-broadcast lhsT
            nc.vector.memset(ones_rowb[:], 1.0)

            # ---------------- reads ----------------
            # ACT ring front: bias col, input group 0, W_in2^T half A.
            # Half B and input group 1 are spliced into the ACT stream
            # later (their ~2.4us descriptor-gen would otherwise delay the
            # first bias ops). SP ring front (before any write): small
            # tensors, memory (early: it heads the DVE critical chain),
            # input groups 2-3, W_mem2^T.
            # ALL DMA issue lives on the SP ring: a big read's descriptor
            # generation costs ~2.4us of sequencer time, which on the ACT
            # ring would serialize ahead of the bias/exp pipeline that the
            # whole softmax chain hangs off. SP only runs DMAs.
            bi2c = wpool.tile([P, KT], F32, tag="bi2c")
            nc.sync.dma_start(bi2c[:], bi2c_d[:])
            w1c = wpool.tile([P, KT], BF16, tag="w1c")
            nc.sync.dma_start(w1c[:], w1c_d[:])
            inTg0 = inpool.tile([P, KT, GL], BF16, tag="inTg0")
            nc.sync.dma_start(
                inTg0[:],
                inT_d[:, 0:GL].rearrange("(k p) x -> p k x", p=P),
            )
            w2half = [wpool.tile([P, KT, GL], BF16, tag=f"w2h{h}",
                                 name=f"w2h{h}") for h in range(2)]
            nc.sync.dma_start(
                w2half[0][:],
                w2t_d[:, 0:GL].rearrange("(k p) x -> p k x", p=P),
            )
            wm1_bc = wpool.tile([P, HID], BF16, tag="wm1bc")
            nc.sync.dma_start(wm1_bc[:], wm1_d.to_broadcast([P, HID]))
            mask_col = wpool.tile([P, MT], F32, tag="maskc")
            nc.sync.dma_start(mask_col[:],
                              mask_d.rearrange("1 (o p) -> p o", p=P))
            bm2c = wpool.tile([P, KT], F32, tag="bm2c")
            nc.sync.dma_start(bm2c[:], bm2c_d[:])

            inTg1 = inpool.tile([P, KT, GL], BF16, tag="inTg1")
            wm2t_sb = wpool.tile([P, KT, HID], BF16, tag="wm2t")
            mem_t = wpool.tile([P, MT, HID], BF16, tag="memt")
            inTg23 = {
                g: inpool.tile([P, KT, GL], BF16, tag=f"inTg{g}",
                               name=f"inTg{g}")
                for g in (2, 3)
            }

            def load_w2halfB():
                nc.sync.dma_start(
                    w2half[1][:],
                    w2t_d[:, GL:2 * GL].rearrange("(k p) x -> p k x", p=P),
                )

            def load_inTg1():
                nc.sync.dma_start(
                    inTg1[:],
                    inT_d[:, GL:2 * GL].rearrange("(k p) x -> p k x", p=P),
                )

            def load_wm2t():
                nc.sync.dma_start(
                    wm2t_sb[:], wm2t_d.rearrange("(k p) d -> p k d", p=P)
                )

            def load_mem():
                nc.sync.dma_start(mem_t[:],
                                  mem_d.rearrange("(j p) d -> p j d", p=P))

            def load_inTg23(g):
                nc.sync.dma_start(
                    inTg23[g][:],
                    inT_d[:, g * GL:(g + 1) * GL].rearrange(
                        "(k p) x -> p k x", p=P),
                )

            def w2ap(oc, k):
                return w2half[oc // 4][:, k, (oc % 4) * P:(oc % 4 + 1) * P]

            def inap(g, k):
                if g == 0:
                    return inTg0[:, k, :]
                if g == 1:
                    return inTg1[:, k, :]
                return inTg23[g][:, k, :]

            # ---------------- persistent state ----------------
            i2T = [i2pool.tile([P, LD], BF16, tag=f"i2_{oc}", name=f"i2_{oc}")
                   for oc in range(KT)]
            o3sb = [o3pool.tile([P, LD], BF16, tag=f"o3_{oc}", name=f"o3_{oc}")
                    for oc in range(KT)]
            id_row = smpool.tile([1, LD], BF16, tag="idrow")
            e_bc = smpool.tile([P, LD], BF16, tag="ebc")
            zacc = smpool.tile([P, GT], F32, tag="zacc")
            z_col = smpool.tile([P, 1], F32, tag="zcol")
            rz_col = smpool.tile([P, 1], F32, tag="rzcol")
            q_part = smpool.tile([P, KT, GT], F32, tag="qpart")
            q_col = smpool.tile([P, KT], F32, tag="qcol")
            q_colb = smpool.tile([P, KT], BF16, tag="qcolb")
            s_mcol = smpool.tile([P, MT], F32, tag="smcol")
            msk = smpool.tile([P, MT], F32, tag="msk")
            e_s = smpool.tile([P, MT], BF16, tag="es")
            es_r = smpool.tile([P, 1], F32, tag="esr")
            zsbc_sb = smpool.tile([P, 1], F32, tag="zsbc")
            rzs_col = smpool.tile([P, 1], F32, tag="rzs")
            p_col = smpool.tile([P, KT], BF16, tag="pcol")
            v_colf = smpool.tile([P, KT], F32, tag="vcolf")
            v_col = smpool.tile([P, KT], F32, tag="vcol")
            sq_col = smpool.tile([P, KT], F32, tag="sqcol")
            u_col = smpool.tile([P, KT], F32, tag="ucol")

            # ---------------- emitters ----------------
            def emit_mm(g, oc):
                ps = psmm.tile([P, GL], F32, tag="mm", name=f"mm{g}_{oc}")
                for k in range(KT):
                    nc.tensor.matmul(
                        ps[:], w2ap(oc, k), inap(g, k),
                        start=(k == 0), stop=(k == KT - 1),
                    )
                nc.scalar.activation(
                    i2T[oc][:, g * GL:(g + 1) * GL], ps[:], IDENT,
                    bias=bi2c[:, oc:oc + 1],
                )
                nc.sync.dma_start(
                    o1T_d[oc * P:(oc + 1) * P, g * GL:(g + 1) * GL],
                    i2T[oc][:, g * GL:(g + 1) * GL],
                )

            def emit_idot(g):
                # idot row for group g: [1, 512] accumulated over k with a
                # [128, 1] stationary — no transposes needed downstream.
                ps = psid.tile([1, GL], F32, tag="idr", name=f"idr{g}")
                for k in range(KT):
                    nc.tensor.matmul(
                        ps[:], w1c[:, k:k + 1], inap(g, k),
                        start=(k == 0), stop=(k == KT - 1),
                    )
                nc.scalar.copy(id_row[:, g * GL:(g + 1) * GL], ps[:])

            def emit_ebc(g):
                ps = psbc.tile([P, GL], F32, tag="bc", name=f"ebc{g}")
                nc.tensor.matmul(
                    ps[:], ones_rowb[:], id_row[:, g * GL:(g + 1) * GL],
                    start=True, stop=True,
                )
                nc.scalar.activation(
                    e_bc[:, g * GL:(g + 1) * GL], ps[:], EXP,
                    accum_out=zacc[:, g:g + 1],
                )

            def emit_q(g, eng):
                # q_part[d, k, g] = sum_{l in g} inT[d,l] * e[l]
                # (GPSIMD offload of the muls corrupts SBUF with bf16
                # operands — everything stays on DVE.)
                del eng
                for k in range(KT):
                    scr = scrpool.tile([P, GL], BF16, tag="scr",
                                       name=f"q{k}_{g}")
                    nc.vector.tensor_mul(
                        scr[:], inap(g, k), e_bc[:, g * GL:(g + 1) * GL]
                    )
                    nc.vector.tensor_reduce(
                        q_part[:, k, g:g + 1], scr[:], AX.X, OP.add
                    )

            def emit_smul(j):
                scrm = scrpool.tile([P, HID], BF16, tag="scrm", name=f"smul{j}")
                nc.vector.tensor_mul(scrm[:], mem_t[:, j, :], wm1_bc[:])
                nc.vector.tensor_reduce(
                    s_mcol[:, j:j + 1], scrm[:], AX.X, OP.add
                )

            def emit_es():
                nc.vector.tensor_scalar(msk[:], mask_col[:], -1.0, 1e30,
                                        OP.add, OP.mult)
                nc.vector.tensor_add(msk[:], msk[:], s_mcol[:])
                nc.scalar.activation(e_s[:], msk[:], EXP)

            def emit_zs():
                # Z_s = sum over all m of e_s, replicated to every
                # partition in one GPSIMD all-reduce (no PE/ACT ping-pong)
                nc.vector.tensor_reduce(es_r[:], e_s[:], AX.X, OP.add)
                nc.gpsimd.partition_all_reduce(zsbc_sb[:], es_r[:], P,
                                               bass_isa.ReduceOp.add)
                nc.vector.reciprocal(rzs_col[:], zsbc_sb[:])

            def emit_p(dc):
                ps = pssm.tile([P, 1], F32, tag="sm", name=f"p{dc}")
                for j in range(MT):
                    nc.tensor.matmul(
                        ps[:], mem_t[:, j, dc * P:(dc + 1) * P],
                        e_s[:, j:j + 1],
                        start=(j == 0), stop=(j == MT - 1),
                    )
                nc.scalar.activation(p_col[:, dc:dc + 1], ps[:], COPY,
                                     scale=rzs_col[:, 0:1])

            def emit_v(oc):
                ps = pssm.tile([P, 1], F32, tag="sm", name=f"v{oc}")
                for k in range(KT):
                    nc.tensor.matmul(
                        ps[:], wm2t_sb[:, k, oc * P:(oc + 1) * P],
                        p_col[:, k:k + 1],
                        start=(k == 0), stop=(k == KT - 1),
                    )
                nc.vector.tensor_copy(v_colf[:, oc:oc + 1], ps[:])

            def emit_vfin():
                nc.vector.tensor_add(v_col[:], v_colf[:], bm2c[:])
                nc.sync.dma_start(vrow_d[:], v_col[:])

            def emit_qfin():
                for k in range(KT):
                    nc.vector.tensor_reduce(
                        q_col[:, k:k + 1], q_part[:, k, :], AX.X, OP.add
                    )
                nc.vector.tensor_copy(q_colb[:], q_col[:])
                nc.vector.tensor_reduce(z_col[:], zacc[:], AX.X, OP.add)
                nc.vector.reciprocal(rz_col[:], z_col[:])

            def emit_s(oc):
                ps = pssm.tile([P, 1], F32, tag="sm", name=f"s{oc}")
                for k in range(KT):
                    nc.tensor.matmul(
                        ps[:], w2ap(oc, k), q_colb[:, k:k + 1],
                        start=(k == 0), stop=(k == KT - 1),
                    )
                nc.vector.tensor_copy(sq_col[:, oc:oc + 1], ps[:])

            def emit_u():
                nc.vector.tensor_scalar(u_col[:], bi2c[:], z_col[:, 0:1],
                                        None, OP.mult)
                nc.vector.tensor_add(u_col[:], u_col[:], sq_col[:])
                nc.vector.tensor_scalar(u_col[:], u_col[:], rz_col[:, 0:1],
                                        None, OP.mult)
                nc.vector.tensor_mul(u_col[:], u_col[:], v_col[:])
                nc.sync.dma_start(urow_d[:], u_col[:])

            def emit_prod(g, oc):
                nc.vector.tensor_scalar(
                    o3sb[oc][:, g * GL:(g + 1) * GL],
                    i2T[oc][:, g * GL:(g + 1) * GL],
                    v_col[:, oc:oc + 1], None, OP.mult,
                )
                if g == 1:
                    nc.sync.dma_start(
                        o3T_d[oc * P:(oc + 1) * P, 0:2 * GL],
                        o3sb[oc][:, 0:2 * GL],
                    )
                elif g == 3:
                    nc.sync.dma_start(
                        o3T_d[oc * P:(oc + 1) * P, 2 * GL:LD],
                        o3sb[oc][:, 2 * GL:LD],
                    )

            # ---------------- schedule ----------------
            # mm tiles t=1..32 (g outer, oc inner); side events spliced at
            # points where their producers (DMA arrivals or earlier events)
            # are guaranteed done so no engine's in-order stream stalls.
            def splice(t):
                if t == 1:
                    load_w2halfB()
                    load_mem()
                if t == 2:
                    load_inTg1()
                    emit_idot(0)
                if t in (3, 4):
                    emit_smul((t - 3) * 2)
                    emit_smul((t - 3) * 2 + 1)
                if t == 4:
                    load_inTg23(2)
                    emit_ebc(0)
                if t == 5:
                    emit_es()
                    emit_zs()
                if t == 6:
                    load_wm2t()
                    emit_q(0, None)
                if t == 8:
                    load_inTg23(3)
                    emit_idot(1)
                if t == 10:
                    emit_ebc(1)
                    for dc in range(4):
                        emit_p(dc)
                if t == 11:
                    for dc in range(4):
                        emit_p(4 + dc)
                    emit_q(1, None)
                if t == 12:
                    emit_idot(2)
                    for oc in range(4):
                        emit_v(oc)
                if t == 13:
                    emit_ebc(2)
                    for oc in range(4):
                        emit_v(4 + oc)
                if t == 14:
                    emit_vfin()
                    emit_q(2, None)
                if t == 16:
                    emit_idot(3)
                if t == 18:
                    emit_ebc(3)
                    for oc in range(KT):
                        emit_prod(0, oc)
                    for oc in range(KT):
                        emit_prod(1, oc)
                if t == 19:
                    emit_q(3, None)
                if t == 20:
                    emit_qfin()
                # prod(2, *) must wait until mm(2, 7) has been emitted (t=24)
                if t == 24:
                    for oc in range(KT):
                        emit_prod(2, oc)
                if t in (25, 26):
                    for oc in range(4):
                        emit_s((t - 25) * 4 + oc)
                if t == 27:
                    emit_u()

            t = 0
            for g in range(GT):
                for oc in range(KT):
                    emit_mm(g, oc)
                    if g == GT - 1:
                        emit_prod(g, oc)
                    t += 1
                    splice(t)

    nc.finalize()
    return nc


def _get_nc():
    global _NC_CACHE
    if _NC_CACHE is None:
        _NC_CACHE = _build_nc()
    return _NC_CACHE


def kernel(**inputs) -> np.ndarray:
    nc = _get_nc()
    bf16 = ml_dtypes.bfloat16

    inp = np.asarray(inputs["input"], np.float32)
    mem = np.asarray(inputs["memory"], np.float32)
    mask = np.asarray(inputs["mask"], np.float32)
    w_in1 = np.asarray(inputs["w_in1"], np.float32).reshape(HID)
    w_mem1 = np.asarray(inputs["w_mem1"], np.float32).reshape(1, HID)
    W_in2 = np.asarray(inputs["W_in2"], np.float32)
    b_in2 = np.asarray(inputs["b_in2"], np.float32).reshape(HID)
    W_mem2 = np.asarray(inputs["W_mem2"], np.float32)
    b_mem2 = np.asarray(inputs["b_mem2"], np.float32).reshape(HID)

    w2t = W_in2.T.astype(bf16)
    wm2t = W_mem2.T.astype(bf16)
    w1c = w_in1.reshape(KT, P).T.astype(bf16)
    wm1 = w_mem1.astype(bf16)
    bi2c = np.ascontiguousarray(b_in2.reshape(KT, P).T)
    bm2c = np.ascontiguousarray(b_mem2.reshape(KT, P).T)

    in_maps = []
    for b in range(N_CORES):
        in_maps.append({
            "inT": inp[b].T.astype(bf16),
            "w2t": w2t,
            "wm2t": wm2t,
            "memory": mem[b].astype(bf16),
            "mask": np.ascontiguousarray(mask[b].reshape(1, LM)),
            "w1c": w1c,
            "wm1": wm1,
            "bi2c": bi2c,
            "bm2c": bm2c,
        })

    res = run_bass_kernel_spmd(nc, in_maps, core_ids=list(range(N_CORES)))

    out = np.empty((BSZ, 4 * LD, HID), np.float32)
    for b in range(N_CORES):
        r = res.results[b]
        out[b, 0:LD] = r["o1T"].T
        v = r["vrow"].T.reshape(HID).astype(np.float32)
        out[b, LD:2 * LD] = v
        out[b, 2 * LD:3 * LD] = r["o3T"].T
        u = r["urow"].T.reshape(HID).astype(np.float32)
        out[b, 3 * LD:4 * LD] = u
    return out
